# revision 12
# baseline (speedup 1.0000x reference)
"""GRU encoder (Keras reset_after=True) on 8 Trainium2 NeuronCores.

Strategy: data-parallel over batch (64 seqs -> 8 cores x 8 seqs). Each core:
  phase A/B: embedding gather (indirect DMA) + PE transpose + input-projection
             GEMM (x @ Wx + b0[+b1_zr]) -> xp scratch in DRAM (t-major pages)
  phase C:   512-step sequential GRU scan. Per step:
               rec = h @ Wh           (f32r matmuls, moving operand = Wh)
               z' = sigmoid(-(xp_z + rec_z))        (z' = 1 - z)
               r  = sigmoid(xp_r + rec_r)
               hh = tanh(xp_h + r * (rec_h + b1_h))
               h  = h + mask_t * z' * (hh - h)
             h is kept in both [8,1024] (batch-major, gate math) and
             transposed [128, 64] (k-chunk-major, matmul stationary) layouts;
             the transpose runs on the PE each step.
No cross-core communication: the scan is fully independent per batch shard,
so nothing sits on the 512-step serial path but local compute.
"""

import numpy as np

import concourse.bass as bass
import concourse.bacc as bacc
import concourse.mybir as mybir
import concourse.tile as tile
from concourse.bass import ds
from concourse.bass_utils import run_bass_kernel_spmd
from concourse.masks import make_identity

V, E, H = 32000, 256, 1024
B, S = 64, 512
NG = 3 * H            # 3072 gate columns (z, r, h)
NCORES = 8
BL = B // NCORES      # 8 sequences per core
ST = BL * S           # 4096 tokens per core
KC = H // 128         # 8 contraction chunks
EC = E // 128         # 2 embedding chunks
F32 = mybir.dt.float32
F32R = mybir.dt.float32r
I32 = mybir.dt.int32
SIG = mybir.ActivationFunctionType.Sigmoid
TANH = mybir.ActivationFunctionType.Tanh


def _r(ap):
    return ap.bitcast(F32R)


def _build(use_bias_h: bool, unroll: int = 8):
    nc = bacc.Bacc()

    emb_d = nc.declare_dram_parameter("emb", [V, E], F32, isOutput=False)
    idx_d = nc.declare_dram_parameter("idx", [128, ST // 128], I32, isOutput=False)
    mask_d = nc.declare_dram_parameter("mask", [BL, S], F32, isOutput=False)
    wx_d = nc.declare_dram_parameter("wx", [128, EC * NG], F32R, isOutput=False)
    wh_d = nc.declare_dram_parameter("wh", [128, KC * NG], F32R, isOutput=False)
    cvec_d = nc.declare_dram_parameter("cvec", [128, NG], F32, isOutput=False)
    b1h_d = nc.declare_dram_parameter("b1h", [BL, H], F32, isOutput=False)
    seq_d = nc.declare_dram_parameter("seq", [S, BL, H], F32, isOutput=True)
    hfin_d = nc.declare_dram_parameter("hfin", [BL, H], F32, isOutput=True)
    xp_d = nc.dram_tensor("xp", [S, BL, NG], F32)

    with tile.TileContext(nc) as tc:
        with tc.tile_pool(name="const", bufs=1) as cpool:
            ident = cpool.tile([128, 128], F32, tag="ident", name="ident")
            make_identity(nc, ident[:])
            mask_sb = cpool.tile([BL, S], F32, tag="mask", name="mask_sb")
            nc.sync.dma_start(out=mask_sb[:], in_=mask_d[:])
            wh_sb = cpool.tile([128, KC * NG], F32R, tag="wh", name="wh_sb")
            nc.sync.dma_start(out=wh_sb[:], in_=wh_d[:])
            if use_bias_h:
                b1h_sb = cpool.tile([BL, H], F32, tag="b1h", name="b1h_sb")
                nc.sync.dma_start(out=b1h_sb[:], in_=b1h_d[:])

            # ------------- phase A/B: gather + transpose + input GEMM -> xp
            with (
                tc.tile_pool(name="sbB", bufs=2) as bpool,
                tc.tile_pool(name="psB", bufs=2, space="PSUM") as ppB,
            ):
                idx_sb = bpool.tile([128, ST // 128], I32, tag="idx", bufs=1,
                                    name="idx_sb")
                nc.sync.dma_start(out=idx_sb[:], in_=idx_d[:])
                wx_sb = bpool.tile([128, EC * NG], F32R, tag="wx", bufs=1,
                                   name="wx_sb")
                nc.sync.dma_start(out=wx_sb[:], in_=wx_d[:])
                cvec_sb = bpool.tile([128, NG], F32, tag="cvec", bufs=1,
                                     name="cvec_sb")
                nc.sync.dma_start(out=cvec_sb[:], in_=cvec_d[:])

                for i in range(ST // 128):         # 32 s-tiles of 128 tokens
                    bb, t0 = i // (S // 128), (i % (S // 128)) * 128
                    x_sb = bpool.tile([128, E], F32, tag="x", name="x_sb")
                    nc.gpsimd.indirect_dma_start(
                        out=x_sb[:],
                        out_offset=None,
                        in_=emb_d[:],
                        in_offset=bass.IndirectOffsetOnAxis(
                            ap=idx_sb[:, i : i + 1], axis=0
                        ),
                    )
                    xt_sb = bpool.tile([128, E], F32R, tag="xt", name="xt_sb")
                    for c in range(EC):
                        pt = ppB.tile([128, 128], F32, tag="pt", name="pt")
                        nc.tensor.transpose(
                            out=pt[:],
                            in_=x_sb[:, c * 128 : (c + 1) * 128],
                            identity=ident[:],
                        )
                        nc.vector.tensor_copy(
                            out=xt_sb[:, c * 128 : (c + 1) * 128], in_=pt[:]
                        )
                    for j in range(NG // 512):     # 6 n-chunks
                        px = ppB.tile([128, 512], F32, tag="px", name="px")
                        for c in range(EC):
                            nc.tensor.matmul(
                                out=px[:],
                                lhsT=xt_sb[:, c * 128 : (c + 1) * 128],
                                rhs=wx_sb[:, c * NG + j * 512 : c * NG + (j + 1) * 512],
                                start=(c == 0),
                                stop=(c == EC - 1),
                            )
                        ox = bpool.tile([128, 512], F32, tag="ox", bufs=3,
                                        name="ox")
                        nc.vector.tensor_add(
                            out=ox[:], in0=px[:],
                            in1=cvec_sb[:, j * 512 : (j + 1) * 512],
                        )
                        nc.sync.dma_start(
                            out=xp_d[t0 : t0 + 128, bb, j * 512 : (j + 1) * 512],
                            in_=ox[:],
                        )

            # ------------- phase C: the scan
            with (
                tc.tile_pool(name="scan", bufs=2) as gpool,
                tc.tile_pool(name="psC", bufs=1, space="PSUM") as rpool,
            ):
                h_cur = gpool.tile([BL, H], F32, tag="h", bufs=1, name="h_cur")
                hT = gpool.tile([128, KC * BL], F32R, tag="hT", bufs=1, name="hT")
                nc.vector.memset(h_cur[:], 0.0)
                ph0 = rpool.tile([128, KC * BL], F32, tag="ph", name="ph0")
                for k in range(KC):
                    nc.tensor.transpose(
                        out=ph0[:, k * BL : (k + 1) * BL],
                        in_=h_cur[:, k * 128 : (k + 1) * 128],
                        identity=ident[:BL, :BL],
                    )
                nc.vector.tensor_copy(out=hT[:], in_=ph0[:])

                def step(iv):
                    xzb = gpool.tile([BL, NG], F32, tag="xzb", bufs=3, name="xzb")
                    nc.gpsimd.dma_start(out=xzb[:], in_=xp_d[ds(iv, 1), :, :])
                    mt = mask_sb[:, ds(iv, 1)]

                    # rec = h @ Wh into 6 psum banks: z0 z1 r0 r1 h0 h1
                    recs = []
                    for j in range(NG // 512):
                        pr = rpool.tile([BL, 512], F32, tag=f"rec{j}",
                                        name=f"rec{j}")
                        recs.append(pr)
                        for k in range(KC):
                            nc.tensor.matmul(
                                out=pr[:],
                                lhsT=hT[:, k * BL : (k + 1) * BL],
                                rhs=wh_sb[:, k * NG + j * 512 : k * NG + (j + 1) * 512],
                                start=(k == 0),
                                stop=(k == KC - 1),
                            )

                    zo = gpool.tile([BL, H], F32, tag="zo", name="zo")
                    rr = gpool.tile([BL, H], F32, tag="rr", name="rr")
                    hh = gpool.tile([BL, H], F32, tag="hh", name="hh")
                    tp = gpool.tile([BL, H], F32, tag="tp", name="tp")
                    u1 = gpool.tile([BL, H], F32, tag="u1", name="u1")
                    for a in range(2):             # 512-wide halves of each gate
                        sl = slice(a * 512, (a + 1) * 512)
                        nc.vector.tensor_add(out=tp[:, sl], in0=recs[a][:],
                                             in1=xzb[:, sl])
                        nc.scalar.activation(out=zo[:, sl], in_=tp[:, sl],
                                             func=SIG, scale=-1.0)
                        nc.vector.tensor_add(
                            out=tp[:, sl], in0=recs[2 + a][:],
                            in1=xzb[:, 1024 + a * 512 : 1024 + (a + 1) * 512],
                        )
                        nc.scalar.activation(out=rr[:, sl], in_=tp[:, sl],
                                             func=SIG)
                        # w = mask * (1-z); u1 = h*(1-w)  -- off critical path
                        nc.vector.tensor_scalar_mul(out=zo[:, sl],
                                                    in0=zo[:, sl], scalar1=mt)
                        nc.vector.tensor_mul(out=tp[:, sl], in0=h_cur[:, sl],
                                             in1=zo[:, sl])
                        nc.vector.tensor_sub(out=u1[:, sl], in0=h_cur[:, sl],
                                             in1=tp[:, sl])
                    ph = rpool.tile([128, KC * BL], F32, tag="ph", name="ph")
                    for a in range(2):
                        sl = slice(a * 512, (a + 1) * 512)
                        rech = recs[4 + a][:]
                        if use_bias_h:
                            nc.vector.tensor_add(out=tp[:, sl], in0=rech,
                                                 in1=b1h_sb[:, sl])
                            rech = tp[:, sl]
                        nc.vector.tensor_mul(out=hh[:, sl], in0=rr[:, sl],
                                             in1=rech)
                        nc.vector.tensor_add(
                            out=hh[:, sl], in0=hh[:, sl],
                            in1=xzb[:, 2048 + a * 512 : 2048 + (a + 1) * 512],
                        )
                        nc.scalar.activation(out=hh[:, sl], in_=hh[:, sl],
                                             func=TANH)
                        # h_new = u1 + w*hh  (2-op tail after tanh)
                        nc.vector.tensor_mul(out=hh[:, sl], in0=zo[:, sl],
                                             in1=hh[:, sl])
                        nc.vector.tensor_add(out=h_cur[:, sl], in0=hh[:, sl],
                                             in1=u1[:, sl])
                        # transpose this half to hT so next step's first
                        # k-chunk matmuls can start before the other half
                        for k in range(4 * a, 4 * a + 4):
                            nc.tensor.transpose(
                                out=ph[:, k * BL : (k + 1) * BL],
                                in_=h_cur[:, k * 128 : (k + 1) * 128],
                                identity=ident[:BL, :BL],
                            )
                        nc.vector.tensor_copy(
                            out=hT[:, a * 4 * BL : (a + 1) * 4 * BL],
                            in_=ph[:, a * 4 * BL : (a + 1) * 4 * BL],
                        )

                    nc.gpsimd.dma_start(out=seq_d[ds(iv, 1), :, :], in_=h_cur[:])

                def unrollable_body(iv0, n_unroll):
                    for jj in range(n_unroll):
                        step(iv0 + jj)

                tc.For_i_unrolled_general(
                    0, S, 1, unrollable_body, max_unroll=unroll,
                    hint_engines=(mybir.EngineType.PE,),
                )

                nc.sync.dma_start(out=hfin_d[:], in_=h_cur[:])

    return nc


_CACHE: dict = {}
TRACE = False          # set by test.py to profile; harness leaves it False
LAST_RESULT = None     # BassKernelResults of the most recent run


def _get_program(use_bias_h: bool):
    key = ("v1", use_bias_h)
    if key not in _CACHE:
        nc = _build(use_bias_h)
        if not nc.is_finalized():
            nc.finalize()
        _CACHE[key] = nc
    return _CACHE[key]


def kernel(tokens, emb, Wx, Wh, b):
    tokens = np.asarray(tokens)
    emb = np.ascontiguousarray(np.asarray(emb, dtype=np.float32))
    Wx = np.asarray(Wx, dtype=np.float32)
    Wh = np.asarray(Wh, dtype=np.float32)
    b = np.asarray(b, dtype=np.float32)

    use_bias_h = bool(np.any(b[1, 2 * H :]))
    nc = _get_program(use_bias_h)

    # host-side layout prep (shard + pre-permute weights; the only math is
    # folding the two bias rows together)
    wx_h = np.ascontiguousarray(
        Wx.reshape(EC, 128, NG).transpose(1, 0, 2).reshape(128, EC * NG)
    )
    wh_h = np.ascontiguousarray(
        Wh.reshape(KC, 128, NG).transpose(1, 0, 2).reshape(128, KC * NG)
    )
    cvec = b[0].copy()
    cvec[: 2 * H] += b[1, : 2 * H]          # z,r absorb both biases; h keeps b0
    cvec128 = np.ascontiguousarray(np.broadcast_to(cvec, (128, NG)))
    b1h = np.ascontiguousarray(np.broadcast_to(b[1, 2 * H :], (BL, H)))

    in_maps = []
    for c in range(NCORES):
        tok = tokens[c * BL : (c + 1) * BL].astype(np.int32)      # [8, 512]
        flat = tok.reshape(-1)                                    # s = b*512 + t
        idx = np.ascontiguousarray(flat.reshape(ST // 128, 128).T).astype(np.int32)
        m = np.ascontiguousarray((tok != 0).astype(np.float32))
        in_maps.append(
            {
                "emb": emb,
                "idx": idx,
                "mask": m,
                "wx": wx_h,
                "wh": wh_h,
                "cvec": cvec128,
                "b1h": b1h,
            }
        )

    try:
        res = run_bass_kernel_spmd(nc, in_maps, list(range(NCORES)), trace=TRACE)
    except ModuleNotFoundError:
        res = run_bass_kernel_spmd(nc, in_maps, list(range(NCORES)))
    global LAST_RESULT
    LAST_RESULT = res

    seq = np.empty((B, S, H), dtype=np.float32)
    hT = np.empty((B, H), dtype=np.float32)
    for c in range(NCORES):
        out = res.results[c]
        seq[c * BL : (c + 1) * BL] = np.asarray(out["seq"]).transpose(1, 0, 2)
        hT[c * BL : (c + 1) * BL] = np.asarray(out["hfin"])
    return seq, hT


# revision 13
# speedup vs baseline: 1.5054x; 1.5054x over previous
"""GRU encoder (Keras reset_after=True) on 8 Trainium2 NeuronCores.

Strategy: data-parallel over batch (64 seqs -> 8 cores x 8 seqs). Each core:
  phase A/B: embedding gather (indirect DMA) + PE transpose + input-projection
             GEMM (x @ Wx + b0[+b1_zr]) -> xp scratch in DRAM (t-major pages)
  phase C:   512-step sequential GRU scan. Per step:
               rec = h @ Wh           (f32r matmuls, moving operand = Wh)
               z' = sigmoid(-(xp_z + rec_z))        (z' = 1 - z)
               r  = sigmoid(xp_r + rec_r)
               hh = tanh(xp_h + r * (rec_h + b1_h))
               h  = h + mask_t * z' * (hh - h)
             h is kept in both [8,1024] (batch-major, gate math) and
             transposed [128, 64] (k-chunk-major, matmul stationary) layouts;
             the transpose runs on the PE each step.
No cross-core communication: the scan is fully independent per batch shard,
so nothing sits on the 512-step serial path but local compute.
"""

import numpy as np

import concourse.bass as bass
import concourse.bacc as bacc
import concourse.mybir as mybir
import concourse.tile as tile
from concourse.bass import ds
from concourse.bass_utils import run_bass_kernel_spmd
from concourse.masks import make_identity

V, E, H = 32000, 256, 1024
B, S = 64, 512
NG = 3 * H            # 3072 gate columns (z, r, h)
NCORES = 8
BL = B // NCORES      # 8 sequences per core
ST = BL * S           # 4096 tokens per core
KC = H // 128         # 8 contraction chunks
EC = E // 128         # 2 embedding chunks
F32 = mybir.dt.float32
F32R = mybir.dt.float32r
I32 = mybir.dt.int32
SIG = mybir.ActivationFunctionType.Sigmoid
TANH = mybir.ActivationFunctionType.Tanh


def _r(ap):
    return ap.bitcast(F32R)


def _build(use_bias_h: bool, unroll: int = 8):
    nc = bacc.Bacc()

    emb_d = nc.declare_dram_parameter("emb", [V, E], F32, isOutput=False)
    idx_d = nc.declare_dram_parameter("idx", [128, ST // 128], I32, isOutput=False)
    mask_d = nc.declare_dram_parameter("mask", [BL, S], F32, isOutput=False)
    wx_d = nc.declare_dram_parameter("wx", [128, EC * NG], F32R, isOutput=False)
    wh_d = nc.declare_dram_parameter("wh", [128, KC * NG], F32R, isOutput=False)
    cvec_d = nc.declare_dram_parameter("cvec", [128, NG], F32, isOutput=False)
    b1h_d = nc.declare_dram_parameter("b1h", [BL, H], F32, isOutput=False)
    seq_d = nc.declare_dram_parameter("seq", [S, BL, H], F32, isOutput=True)
    hfin_d = nc.declare_dram_parameter("hfin", [BL, H], F32, isOutput=True)
    xp_d = nc.dram_tensor("xp", [S, BL, NG], F32)

    with tile.TileContext(nc) as tc:
        with tc.tile_pool(name="const", bufs=1) as cpool:
            ident = cpool.tile([128, 128], F32, tag="ident", name="ident")
            make_identity(nc, ident[:])
            mask_sb = cpool.tile([BL, S], F32, tag="mask", name="mask_sb")
            nc.sync.dma_start(out=mask_sb[:], in_=mask_d[:])
            wh_sb = cpool.tile([128, KC * NG], F32R, tag="wh", name="wh_sb")
            nc.sync.dma_start(out=wh_sb[:], in_=wh_d[:])
            if use_bias_h:
                b1h_sb = cpool.tile([BL, H], F32, tag="b1h", name="b1h_sb")
                nc.sync.dma_start(out=b1h_sb[:], in_=b1h_d[:])

            # ------------- phase A/B: gather + transpose + input GEMM -> xp
            with (
                tc.tile_pool(name="sbB", bufs=2) as bpool,
                tc.tile_pool(name="psB", bufs=2, space="PSUM") as ppB,
            ):
                idx_sb = bpool.tile([128, ST // 128], I32, tag="idx", bufs=1,
                                    name="idx_sb")
                nc.sync.dma_start(out=idx_sb[:], in_=idx_d[:])
                wx_sb = bpool.tile([128, EC * NG], F32R, tag="wx", bufs=1,
                                   name="wx_sb")
                nc.sync.dma_start(out=wx_sb[:], in_=wx_d[:])
                cvec_sb = bpool.tile([128, NG], F32, tag="cvec", bufs=1,
                                     name="cvec_sb")
                nc.sync.dma_start(out=cvec_sb[:], in_=cvec_d[:])

                for i in range(ST // 128):         # 32 s-tiles of 128 tokens
                    bb, t0 = i // (S // 128), (i % (S // 128)) * 128
                    x_sb = bpool.tile([128, E], F32, tag="x", name="x_sb")
                    nc.gpsimd.indirect_dma_start(
                        out=x_sb[:],
                        out_offset=None,
                        in_=emb_d[:],
                        in_offset=bass.IndirectOffsetOnAxis(
                            ap=idx_sb[:, i : i + 1], axis=0
                        ),
                    )
                    xt_sb = bpool.tile([128, E], F32R, tag="xt", name="xt_sb")
                    for c in range(EC):
                        pt = ppB.tile([128, 128], F32, tag="pt", name="pt")
                        nc.tensor.transpose(
                            out=pt[:],
                            in_=x_sb[:, c * 128 : (c + 1) * 128],
                            identity=ident[:],
                        )
                        nc.vector.tensor_copy(
                            out=xt_sb[:, c * 128 : (c + 1) * 128], in_=pt[:]
                        )
                    for j in range(NG // 512):     # 6 n-chunks
                        px = ppB.tile([128, 512], F32, tag="px", name="px")
                        for c in range(EC):
                            nc.tensor.matmul(
                                out=px[:],
                                lhsT=xt_sb[:, c * 128 : (c + 1) * 128],
                                rhs=wx_sb[:, c * NG + j * 512 : c * NG + (j + 1) * 512],
                                start=(c == 0),
                                stop=(c == EC - 1),
                            )
                        ox = bpool.tile([128, 512], F32, tag="ox", bufs=3,
                                        name="ox")
                        nc.vector.tensor_add(
                            out=ox[:], in0=px[:],
                            in1=cvec_sb[:, j * 512 : (j + 1) * 512],
                        )
                        nc.sync.dma_start(
                            out=xp_d[t0 : t0 + 128, bb, j * 512 : (j + 1) * 512],
                            in_=ox[:],
                        )

            # ------------- phase C: the scan
            with (
                tc.tile_pool(name="scan", bufs=2) as gpool,
                tc.tile_pool(name="psC", bufs=1, space="PSUM") as rpool,
            ):
                h_cur = gpool.tile([BL, H], F32, tag="h", bufs=1, name="h_cur")
                hT = gpool.tile([128, KC * BL], F32R, tag="hT", bufs=1, name="hT")
                nc.vector.memset(h_cur[:], 0.0)
                ph0 = rpool.tile([128, KC * BL], F32, tag="ph", name="ph0")
                for k in range(KC):
                    nc.tensor.transpose(
                        out=ph0[:, k * BL : (k + 1) * BL],
                        in_=h_cur[:, k * 128 : (k + 1) * 128],
                        identity=ident[:BL, :BL],
                    )
                nc.vector.tensor_copy(out=hT[:], in_=ph0[:])

                def step(iv):
                    xzb = gpool.tile([BL, NG], F32, tag="xzb", bufs=3, name="xzb")
                    nc.gpsimd.dma_start(out=xzb[:], in_=xp_d[ds(iv, 1), :, :])
                    mt = mask_sb[:, ds(iv, 1)]

                    # rec = h @ Wh into 6 psum banks: z0 z1 r0 r1 h0 h1
                    recs = []
                    for j in range(NG // 512):
                        pr = rpool.tile([BL, 512], F32, tag=f"rec{j}",
                                        name=f"rec{j}")
                        recs.append(pr)
                        for k in range(KC):
                            nc.tensor.matmul(
                                out=pr[:],
                                lhsT=hT[:, k * BL : (k + 1) * BL],
                                rhs=wh_sb[:, k * NG + j * 512 : k * NG + (j + 1) * 512],
                                start=(k == 0),
                                stop=(k == KC - 1),
                            )

                    zo = gpool.tile([BL, H], F32, tag="zo", name="zo")
                    rr = gpool.tile([BL, H], F32, tag="rr", name="rr")
                    hh = gpool.tile([BL, H], F32, tag="hh", name="hh")
                    tp = gpool.tile([BL, H], F32, tag="tp", name="tp")
                    u1 = gpool.tile([BL, H], F32, tag="u1", name="u1")
                    for a in range(2):             # 512-wide halves of each gate
                        sl = slice(a * 512, (a + 1) * 512)
                        nc.vector.tensor_add(out=tp[:, sl], in0=recs[a][:],
                                             in1=xzb[:, sl])
                        nc.scalar.activation(out=zo[:, sl], in_=tp[:, sl],
                                             func=SIG, scale=-1.0)
                        nc.vector.tensor_add(
                            out=tp[:, sl], in0=recs[2 + a][:],
                            in1=xzb[:, 1024 + a * 512 : 1024 + (a + 1) * 512],
                        )
                        nc.scalar.activation(out=rr[:, sl], in_=tp[:, sl],
                                             func=SIG)
                        # w = mask * (1-z); u1 = h*(1-w)  -- off critical path
                        nc.vector.tensor_scalar_mul(out=zo[:, sl],
                                                    in0=zo[:, sl], scalar1=mt)
                        nc.vector.tensor_mul(out=tp[:, sl], in0=h_cur[:, sl],
                                             in1=zo[:, sl])
                        nc.vector.tensor_sub(out=u1[:, sl], in0=h_cur[:, sl],
                                             in1=tp[:, sl])
                    ph = rpool.tile([128, KC * BL], F32, tag="ph", name="ph")
                    for a in range(2):
                        sl = slice(a * 512, (a + 1) * 512)
                        rech = recs[4 + a][:]
                        if use_bias_h:
                            nc.vector.tensor_add(out=tp[:, sl], in0=rech,
                                                 in1=b1h_sb[:, sl])
                            rech = tp[:, sl]
                        nc.vector.tensor_mul(out=hh[:, sl], in0=rr[:, sl],
                                             in1=rech)
                        nc.vector.tensor_add(
                            out=hh[:, sl], in0=hh[:, sl],
                            in1=xzb[:, 2048 + a * 512 : 2048 + (a + 1) * 512],
                        )
                        nc.scalar.activation(out=hh[:, sl], in_=hh[:, sl],
                                             func=TANH)
                        # h_new = u1 + w*hh  (2-op tail after tanh)
                        nc.vector.tensor_mul(out=hh[:, sl], in0=zo[:, sl],
                                             in1=hh[:, sl])
                        nc.vector.tensor_add(out=h_cur[:, sl], in0=hh[:, sl],
                                             in1=u1[:, sl])
                        # transpose this half to hT so next step's first
                        # k-chunk matmuls can start before the other half
                        for k in range(4 * a, 4 * a + 4):
                            nc.tensor.transpose(
                                out=ph[:, k * BL : (k + 1) * BL],
                                in_=h_cur[:, k * 128 : (k + 1) * 128],
                                identity=ident[:BL, :BL],
                            )
                        nc.vector.tensor_copy(
                            out=hT[:, a * 4 * BL : (a + 1) * 4 * BL],
                            in_=ph[:, a * 4 * BL : (a + 1) * 4 * BL],
                        )

                    nc.gpsimd.dma_start(out=seq_d[ds(iv, 1), :, :], in_=h_cur[:])

                with tc.For_i(
                    0, S, unroll,
                    hint_engines=(mybir.EngineType.PE,),
                    staggered_reset=True,
                ) as iv0:
                    for jj in range(unroll):
                        step(iv0 + jj)

                nc.sync.dma_start(out=hfin_d[:], in_=h_cur[:])

    return nc


_CACHE: dict = {}
TRACE = False          # set by test.py to profile; harness leaves it False
LAST_RESULT = None     # BassKernelResults of the most recent run


def _get_program(use_bias_h: bool):
    key = ("v1", use_bias_h)
    if key not in _CACHE:
        nc = _build(use_bias_h)
        if not nc.is_finalized():
            nc.finalize()
        _CACHE[key] = nc
    return _CACHE[key]


def kernel(tokens, emb, Wx, Wh, b):
    tokens = np.asarray(tokens)
    emb = np.ascontiguousarray(np.asarray(emb, dtype=np.float32))
    Wx = np.asarray(Wx, dtype=np.float32)
    Wh = np.asarray(Wh, dtype=np.float32)
    b = np.asarray(b, dtype=np.float32)

    use_bias_h = bool(np.any(b[1, 2 * H :]))
    nc = _get_program(use_bias_h)

    # host-side layout prep (shard + pre-permute weights; the only math is
    # folding the two bias rows together)
    wx_h = np.ascontiguousarray(
        Wx.reshape(EC, 128, NG).transpose(1, 0, 2).reshape(128, EC * NG)
    )
    wh_h = np.ascontiguousarray(
        Wh.reshape(KC, 128, NG).transpose(1, 0, 2).reshape(128, KC * NG)
    )
    cvec = b[0].copy()
    cvec[: 2 * H] += b[1, : 2 * H]          # z,r absorb both biases; h keeps b0
    cvec128 = np.ascontiguousarray(np.broadcast_to(cvec, (128, NG)))
    b1h = np.ascontiguousarray(np.broadcast_to(b[1, 2 * H :], (BL, H)))

    in_maps = []
    for c in range(NCORES):
        tok = tokens[c * BL : (c + 1) * BL].astype(np.int32)      # [8, 512]
        flat = tok.reshape(-1)                                    # s = b*512 + t
        idx = np.ascontiguousarray(flat.reshape(ST // 128, 128).T).astype(np.int32)
        m = np.ascontiguousarray((tok != 0).astype(np.float32))
        in_maps.append(
            {
                "emb": emb,
                "idx": idx,
                "mask": m,
                "wx": wx_h,
                "wh": wh_h,
                "cvec": cvec128,
                "b1h": b1h,
            }
        )

    try:
        res = run_bass_kernel_spmd(nc, in_maps, list(range(NCORES)), trace=TRACE)
    except ModuleNotFoundError:
        res = run_bass_kernel_spmd(nc, in_maps, list(range(NCORES)))
    global LAST_RESULT
    LAST_RESULT = res

    seq = np.empty((B, S, H), dtype=np.float32)
    hT = np.empty((B, H), dtype=np.float32)
    for c in range(NCORES):
        out = res.results[c]
        seq[c * BL : (c + 1) * BL] = np.asarray(out["seq"]).transpose(1, 0, 2)
        hT[c * BL : (c + 1) * BL] = np.asarray(out["hfin"])
    return seq, hT
